# revision 23
# baseline (speedup 1.0000x reference)
"""BiMambaBlock Trainium2 kernel.

Data-parallel over batch: BPC batches per core on NCORES cores
(NCORES * BPC = 8). Each core runs the full bidirectional Mamba block
per batch in one Bass program:

  - middle-section layout: [d_inner on partitions, time on free]
  - projections (in_proj / x_proj / dt_proj / out_proj) as PE GEMMs (fp16)
  - causal depthwise conv: fused mult-add (scalar_tensor_tensor) tap chain
  - silu / softplus fused into single ACT ops (AF.Silu, exp+ln)
  - selective scan via DVE tensor_tensor_scan, one lane per (d, n) pair;
    backward direction scans reversed-time APs
  - dBx/hC elementwise multiplies on GPSIMD (~1 in 5 dBx shifted to DVE
    to balance the two engines)
  - n-fold (sum_n C_n * h_n) via identity-matmul PSUM accumulation on PE
  - final combine + layernorm in [time on partitions, d_model on free]

I/O strategy: all weights/consts are baked into the executable as
HLO constants (nc.inline_tensor) so the axon PJRT tunnel does not
re-ship them on every execute; per-run traffic is one fp16 input
(x transposed, [bpc*D, L]) and one fp16 output ([bpc*L, D]). The
[L, D]-layout x needed for the residual+layernorm epilogue is
reconstructed on device by PE-transposing xb. All 16-bit tensors are
fp16 (not bf16): same size and engine rates, 3 more mantissa bits
(rel err 7e-4 vs 5e-3). The program cache is keyed on the weight
bytes, so a harness calling kernel() with different weights rebuilds
correctly.
"""

import hashlib
import sys

sys.path.insert(0, "/opt/trn_rl_repo")

import numpy as np

import concourse.bass as bass
import concourse.mybir as mybir
import concourse.tile as tile
from concourse import bacc

F32 = mybir.dt.float32
FP16 = mybir.dt.float16
AF = mybir.ActivationFunctionType
OP = mybir.AluOpType

B, L, D, DI, NST, RNK, KCONV = 8, 1024, 512, 1024, 16, 32, 4
LN_EPS = 1e-5
NB = DI // 128  # 8 d-blocks
TT = L // 128  # 8 time tiles
TCH = L // 512  # 2 matmul free chunks
PAD = KCONV - 1

BC_MODE = "dma"  # B/C broadcast: "dma" (partition-broadcast DMA) or "pe" (selector matmul)
NQ = 2  # n-steps per B/C broadcast DMA (dma mode)
DA_BUFS, DBX_BUFS, H_BUFS, HC_BUFS = 2, 3, 3, 2  # phase-C chain pipeline depths

NCORES = 4  # cores used (4 x bpc=2 beats 8 x bpc=1: the tunnel's per-core
# dispatch cost is strongly super-linear in core count, ~1.1 ms at 4 cores vs
# ~3.9 ms at 8, while the second batch per core adds only ~1.7 ms device time)
BPC = B // NCORES  # batches per core

nbf = np.float16


def _mk_layout(entries):
    off, lay = 0, {}
    for name, shape in entries:
        n = int(np.prod(shape))
        lay[name] = (off, shape)
        off += n
    return lay, off


WLAY, WN = _mk_layout(
    [(pre + nm, shp)
     for pre in ("f_", "b_")
     for nm, shp in (
         ("w_inT", (D, 2 * DI)),
         ("w_xT", (DI, 64)),
         ("w_dtT", (RNK, DI)),
         ("w_outT", (DI, D)),
     )]
    + [("ident", (128, 128)), ("bsel", (64, 128 * 2 * NST))]
)
CLAY, CN = _mk_layout(
    [(pre + nm, shp)
     for pre in ("f_", "b_")
     for nm, shp in (
         ("conv_w", (DI, KCONV)),
         ("conv_b", (DI, 1)),
         ("dt_b", (DI, 1)),
         ("A", (DI, NST)),
         ("Dv", (DI, 1)),
     )]
    + [("ln_g", (1, D)), ("ln_b", (1, D))]
)


class P:
    """Pool/handle bag shared by the phase builders."""


def _wap(p, name, r0, r1):
    """AP for rows [r0:r1) of packed weight tensor `name`."""
    off, (rows, cols) = WLAY[name]
    return p.wpack[off + r0 * cols: off + r1 * cols].rearrange(
        "(a b) -> a b", a=r1 - r0)


def _cap(p, name, r0, r1):
    off, (rows, cols) = CLAY[name]
    return p.cpack[off + r0 * cols: off + r1 * cols].rearrange(
        "(a b) -> a b", a=r1 - r0)


def _load_dir_consts(nc, p, pre):
    s_pool = p.s_pool
    h = {}
    h["conv_w"] = [s_pool.tile([128, KCONV], F32, tag=f"conv_w{pre}{m}", name=f"conv_w{pre}{m}") for m in range(NB)]
    h["conv_b"] = [s_pool.tile([128, 1], F32, tag=f"conv_b{pre}{m}", name=f"conv_b{pre}{m}") for m in range(NB)]
    h["dt_b"] = [s_pool.tile([128, 1], F32, tag=f"dt_b{pre}{m}", name=f"dt_b{pre}{m}") for m in range(NB)]
    h["a_sb"] = [s_pool.tile([128, NST], F32, tag=f"a_sb{pre}{m}", name=f"a_sb{pre}{m}") for m in range(NB)]
    h["dv"] = [s_pool.tile([128, 1], F32, tag=f"dv{pre}{m}", name=f"dv{pre}{m}") for m in range(NB)]
    for m in range(NB):
        r0, r1 = 128 * m, 128 * (m + 1)
        nc.sync.dma_start(h["conv_w"][m][:], _cap(p, pre + "conv_w", r0, r1))
        nc.sync.dma_start(h["conv_b"][m][:], _cap(p, pre + "conv_b", r0, r1))
        nc.sync.dma_start(h["dt_b"][m][:], _cap(p, pre + "dt_b", r0, r1))
        nc.sync.dma_start(h["a_sb"][m][:], _cap(p, pre + "A", r0, r1))
        nc.sync.dma_start(h["dv"][m][:], _cap(p, pre + "Dv", r0, r1))
    return h


def _phase_a(nc, p, pre, rev, bi):
    """in_proj GEMM; z -> silu(z) -> DRAM; xi -> causal conv -> silu -> xc."""
    w_in = [p.w_pool.tile([128, 2 * DI], FP16, tag=f"w_in{k}", name=f"w_in{k}") for k in range(4)]
    for k in range(4):
        nc.sync.dma_start(w_in[k][:], _wap(p, pre + "w_inT", 128 * k, 128 * (k + 1)))
    con = p.consts[pre]

    xc = [p.big_pool.tile([128, L], FP16, tag=f"{pre}xc{m}", name=f"{pre}xc{m}") for m in range(NB)]
    sz_dram = p.sz_dram[pre]

    # z tiles first (keeps all silu ACT ops before any exp/ln ACT ops)
    for m in range(2 * NB):
        mm = m + NB if m < NB else m - NB  # z tiles (8..15) first, then xi (0..7)
        xi_pad = None
        sz_st = None
        if mm < NB:
            xi_pad = p.work_pool.tile([128, L + PAD], FP16, tag="xi_pad", name="xi_pad", bufs=2)
            if rev:
                nc.vector.memset(xi_pad[:, L:L + PAD], 0.0)
            else:
                nc.vector.memset(xi_pad[:, 0:PAD], 0.0)
        else:
            sz_st = p.work_pool.tile([128, L], FP16, tag="sz_st", name="sz_st", bufs=2)
        for tch in range(TCH):
            ps = p.ps_pool.tile([128, 512], F32, tag="mm", name="mm")
            for k in range(4):
                nc.tensor.matmul(
                    ps[:],
                    w_in[k][:, 128 * mm:128 * (mm + 1)],
                    p.xT[k][:, 512 * tch:512 * (tch + 1)],
                    start=(k == 0),
                    stop=(k == 3),
                )
            if mm < NB:
                off = (0 if rev else PAD) + 512 * tch
                nc.scalar.activation(xi_pad[:, off:off + 512], ps[:], AF.Copy)
            else:
                # silu(z) fused on ACT straight from PSUM
                p.sig_insts.append(
                    nc.scalar.activation(
                        sz_st[:, 512 * tch:512 * (tch + 1)], ps[:], AF.Silu
                    )
                )
        if mm < NB:
            # conv: fwd out[t] = sum_j w_j*xi[t-3+j]; bwd out[t] = sum_j w_j*xi[t+3-j]
            cw = con["conv_w"][mm]
            cb = con["conv_b"][mm]
            offs = [3 - j for j in range(KCONV)] if rev else list(range(KCONV))
            acc = [
                p.work_pool.tile([128, L], FP16, tag=f"cacc{j % 2}", name=f"cacc{j}", bufs=2)
                for j in range(2)
            ]
            # tap 0 on ACT (Copy with scale-ptr); taps 1-3 as fused mult-add on DVE
            nc.scalar.activation(
                acc[0][:], xi_pad[:, offs[0]:offs[0] + L], AF.Copy, scale=cw[:, 0:1]
            )
            for j in range(1, KCONV):
                o = offs[j]
                nc.vector.scalar_tensor_tensor(
                    acc[j % 2][:], xi_pad[:, o:o + L], cw[:, j:j + 1], acc[(j - 1) % 2][:],
                    OP.mult, OP.add,
                )
            # xc = silu(acc + conv_b) fused on ACT
            p.sig_insts.append(
                nc.scalar.activation(xc[mm][:], acc[(KCONV - 1) % 2][:], AF.Silu, bias=cb[:, 0:1])
            )
        else:
            nc.sync.dma_start(sz_dram[128 * (mm - NB):128 * (mm - NB + 1), :], sz_st[:])
    return {"xc": xc}


def _phase_bcd(nc, p, pre, rev, ten, emit_out, bi):
    xc = ten["xc"]
    con = p.consts[pre]
    sz_dram = p.sz_dram[pre]
    w_x = [p.w_pool.tile([128, 64], FP16, tag=f"w_x{k}", name=f"w_x{k}") for k in range(NB)]
    for k in range(NB):
        nc.sync.dma_start(w_x[k][:], _wap(p, pre + "w_xT", 128 * k, 128 * (k + 1)))
    w_dt = p.w_pool.tile([RNK, DI], FP16, tag="w_dt", name="w_dt")
    nc.sync.dma_start(w_dt[:], _wap(p, pre + "w_dtT", 0, RNK))
    w_out = [p.w_pool.tile([128, D], FP16, tag=f"w_out{k}", name=f"w_out{k}") for k in range(NB)]
    for k in range(NB):
        nc.sync.dma_start(w_out[k][:], _wap(p, pre + "w_outT", 128 * k, 128 * (k + 1)))

    # --- phase B: x_proj -> (dt | B | C); dt_proj -> delta ---
    dbl = p.big_pool.tile([64, L], FP16, tag=f"{pre}dbl", name=f"{pre}dbl")
    for tch in range(TCH):
        ps = p.ps_pool.tile([64, 512], F32, tag="mm", name="mm")
        for k in range(NB):
            nc.tensor.matmul(
                ps[:], w_x[k][:], xc[k][:, 512 * tch:512 * (tch + 1)],
                start=(k == 0), stop=(k == NB - 1),
            )
        nc.scalar.activation(dbl[:, 512 * tch:512 * (tch + 1)], ps[:], AF.Copy)
    if BC_MODE == "dma":
        # bc_dram[n, 0, :] = B_n, bc_dram[n, 1, :] = C_n
        bc_dram = p.dram_pool.tile([NST, 2, L], FP16, tag=f"bc_dram{pre}{bi % 2}",
                                   name="bc_dram")
        nc.sync.dma_start(bc_dram[:, 0, :], dbl[RNK:RNK + NST, :])
        nc.sync.dma_start(bc_dram[:, 1, :], dbl[RNK + NST:RNK + 2 * NST, :])

    delta = [p.big_pool.tile([128, L], FP16, tag=f"delta{m}", name=f"delta{m}") for m in range(NB)]
    for m in range(NB):
        for tch in range(TCH):
            ps = p.ps_pool.tile([128, 512], F32, tag="mm", name="mm")
            nc.tensor.matmul(
                ps[:],
                w_dt[:, 128 * m:128 * (m + 1)],
                dbl[0:RNK, 512 * tch:512 * (tch + 1)],
                start=True, stop=True,
            )
            # softplus(s) = ln(1 + e^s) via the exp/ln table set
            spu = p.work_pool.tile([128, 512], F32, tag="spu", name="spu", bufs=1)
            nc.scalar.activation(spu[:], ps[:], AF.Exp, bias=con["dt_b"][m][:, 0:1])
            nc.scalar.activation(
                delta[m][:, 512 * tch:512 * (tch + 1)], spu[:], AF.Ln, bias=1.0
            )

    # --- phase C: selective scan + n-fold + gate ---
    yg = [p.big_pool.tile([128, L], FP16, tag=f"yg{m}", name=f"yg{m}") for m in range(NB)]
    for g in range(NB // 2):
        yp = [p.psy_pool.tile([128, L], F32, tag=f"yp{d2}", name=f"yp{d2}") for d2 in range(2)]
        dtx = [
            p.work_pool.tile([128, L], FP16, tag=f"cacc{d2}", name=f"dtx{d2}", bufs=2)
            for d2 in range(2)
        ]
        for d2 in range(2):
            m = 2 * g + d2
            nc.gpsimd.tensor_tensor(dtx[d2][:], delta[m][:], xc[m][:], OP.mult)
        def emit_bc(n):
            # broadcast B_n / C_n rows of dbl across 128 partitions on PE
            # (selector matmul), then ACT-copy PSUM -> SBUF.
            bcBt = p.w_pool.tile([128, L], FP16, tag="bcB", name="bcB", bufs=2)
            bcCt = p.w_pool.tile([128, L], FP16, tag="bcC", name="bcC", bufs=2)
            for dst, b in ((bcBt, n), (bcCt, NST + n)):
                for tch in range(TCH):
                    psb = p.ps_pool.tile([128, 512], F32, tag="mm", name="mm")
                    nc.tensor.matmul(
                        psb[:],
                        p.bsel[:, 128 * b:128 * (b + 1)],
                        dbl[0:64, 512 * tch:512 * (tch + 1)],
                        start=True, stop=True,
                    )
                    nc.scalar.activation(
                        dst[:, 512 * tch:512 * (tch + 1)], psb[:], AF.Copy
                    )
            return bcBt, bcCt

        if BC_MODE == "pe":
            bc_cur = emit_bc(0)
        bcq = None
        for n in range(NST):
            if BC_MODE == "pe":
                # prefetch next n's broadcast BEFORE this n's consumers so the
                # in-order PE/ACT streams issue it ahead of the yp-fold stall
                bc_next = emit_bc(n + 1) if n + 1 < NST else None
                bcB = bc_cur[0][:]
                bcC = bc_cur[1][:]
            else:
                if n % NQ == 0:
                    bcq = p.w_pool.tile(
                        [128, NQ, 2, L], FP16, tag=f"bc{(n // NQ) % 2}", name="bc",
                        bufs=1,
                    )
                    nc.sync.dma_start(
                        bcq[:], bc_dram[n:n + NQ, :, :].partition_broadcast(128)
                    )
                bcB = bcq[:, n % NQ, 0, :]
                bcC = bcq[:, n % NQ, 1, :]
            for d2 in range(2):
                m = 2 * g + d2
                da = p.work_pool.tile([128, L], FP16, tag=f"da{d2}", name="da", bufs=DA_BUFS)
                nc.scalar.activation(
                    da[:], delta[m][:], AF.Exp, scale=con["a_sb"][m][:, n:n + 1]
                )
                dbx = p.work_pool.tile([128, L], FP16, tag=f"dbx{d2}", name="dbx", bufs=DBX_BUFS)
                # GPSIMD carries the dbx/hc multiplies, but it is the busiest
                # engine (457us vs DVE 387us); shifting ~1 in 5 dbx ops to DVE
                # balances the two at ~416us each.
                dbx_eng = nc.vector if (g * NST + n) % 5 == 0 else nc.gpsimd
                dbx_eng.tensor_tensor(dbx[:], dtx[d2][:], bcB, OP.mult)
                h = p.work_pool.tile([128, L], FP16, tag=f"h{d2}", name="h", bufs=H_BUFS)
                if rev:
                    nc.vector.tensor_tensor_scan(
                        h[:, ::-1], da[:, ::-1], dbx[:, ::-1], 0.0, OP.mult, OP.add
                    )
                else:
                    nc.vector.tensor_tensor_scan(h[:], da[:], dbx[:], 0.0, OP.mult, OP.add)
                hc = p.work_pool.tile([128, L], FP16, tag=f"hc{d2}", name="hc", bufs=HC_BUFS)
                nc.gpsimd.tensor_tensor(hc[:], h[:], bcC, OP.mult)
                for tch in range(TCH):
                    nc.tensor.matmul(
                        yp[d2][:, 512 * tch:512 * (tch + 1)],
                        p.ident[:],
                        hc[:, 512 * tch:512 * (tch + 1)],
                        start=(n == 0), stop=(n == NST - 1),
                    )
            if BC_MODE == "pe":
                bc_cur = bc_next
        # gate: yg = (y + xc*Dv) * silu(z);  silu(z) streamed back from DRAM
        for d2 in range(2):
            m = 2 * g + d2
            szin = p.io_pool.tile([128, L], FP16, tag=f"szin{d2}", name="szin")
            nc.sync.dma_start(szin[:], sz_dram[128 * m:128 * (m + 1), :])
            for tch in range(TCH):
                nc.vector.scalar_tensor_tensor(
                    yg[m][:, 512 * tch:512 * (tch + 1)],
                    xc[m][:, 512 * tch:512 * (tch + 1)],
                    con["dv"][m][:, 0:1],
                    yp[d2][:, 512 * tch:512 * (tch + 1)],
                    OP.mult, OP.add,
                )
            nc.vector.tensor_tensor(yg[m][:], yg[m][:], szin[:], OP.mult)

    # --- phase D: out_proj GEMM -> [t, D] PSUM tiles ---
    for m in range(TT):
        po = p.psd_pool.tile([128, D], F32, tag="po", name="po")
        for k in range(NB):
            nc.tensor.matmul(
                po[:], yg[k][:, 128 * m:128 * (m + 1)], w_out[k][:],
                start=(k == 0), stop=(k == NB - 1),
            )
        emit_out(m, po)


def build_program(wpack_np, cpack_np, bpc=BPC):
    nc = bacc.Bacc("TRN2", target_bir_lowering=False, debug=False)

    # Force exp/ln onto the one table set that has BOTH, so softplus
    # (exp then ln) doesn't ping-pong table loads. List order (= set ids)
    # is preserved; we only hide exp/ln from the other sets.
    import concourse.bacc as _bacc_mod
    from concourse.hw_specs import get_activation_tables as _gat

    def _patched_tables():
        tables = list(_gat(nc.m.arch).items())
        out = []
        for name, s in tables:
            if name != "natural_log_exp_and_others":
                s = s - {AF.Exp, AF.Ln}
            out.append((name, s))
        _bacc_mod._bass_rust.insert_act_table_loads(nc, out)

    nc.insert_act_table_loads = _patched_tables

    wpack_d = nc.inline_tensor(wpack_np, "wpack")
    cpack_d = nc.inline_tensor(cpack_np, "cpack")
    xb_d = nc.dram_tensor("xb", [bpc * D, L], FP16, kind="ExternalInput")
    out_d = nc.dram_tensor("out", [bpc * L, D], FP16, kind="ExternalOutput")

    with tile.TileContext(nc) as tc:
        with (
            tc.tile_pool(name="io", bufs=1) as io_pool,
            tc.tile_pool(name="w", bufs=1) as w_pool,
            tc.tile_pool(name="big", bufs=1) as big_pool,
            tc.tile_pool(name="work", bufs=2) as work_pool,
            tc.tile_pool(name="s", bufs=1) as s_pool,
            tc.tile_pool(name="ps", bufs=2, space="PSUM") as ps_pool,
            tc.tile_pool(name="psy", bufs=1, space="PSUM") as psy_pool,
            tc.tile_pool(name="psd", bufs=2, space="PSUM") as psd_pool,
            tc.tile_pool(name="dram", bufs=1, space="DRAM") as dram_pool,
        ):
            p = P()
            p.io_pool, p.w_pool, p.big_pool, p.work_pool, p.s_pool = (
                io_pool, w_pool, big_pool, work_pool, s_pool)
            p.ps_pool, p.psy_pool, p.psd_pool, p.dram_pool = (
                ps_pool, psy_pool, psd_pool, dram_pool)
            p.wpack, p.cpack = wpack_d, cpack_d

            p.ident = io_pool.tile([128, 128], FP16, tag="ident", name="ident")
            nc.sync.dma_start(p.ident[:], _wap(p, "ident", 0, 128))
            # broadcast selector: sel[k, 128*b + i] = 1 iff k == 32 + b, so
            # sel[:, 128b:128(b+1)].T @ dbl[0:64, :] replicates dbl row 32+b
            # (B_n for b = n, C_n for b = 16 + n) across all 128 partitions.
            if BC_MODE == "pe":
                p.bsel = io_pool.tile([64, 128 * 2 * NST], FP16, tag="bsel", name="bsel")
                nc.sync.dma_start(p.bsel[:], _wap(p, "bsel", 0, 64))
            g_rep = io_pool.tile([128, 1, D], F32, tag="g_rep", name="g_rep")
            bb_rep = io_pool.tile([128, 1, D], F32, tag="bb_rep", name="bb_rep")
            nc.sync.dma_start(g_rep[:], _cap(p, "ln_g", 0, 1).partition_broadcast(128))
            nc.sync.dma_start(bb_rep[:], _cap(p, "ln_b", 0, 1).partition_broadcast(128))
            eps_t = s_pool.tile([128, 1], F32, tag="eps_t", name="eps_t")
            nc.gpsimd.memset(eps_t[:], LN_EPS)

            p.consts = {
                "f_": _load_dir_consts(nc, p, "f_"),
                "b_": _load_dir_consts(nc, p, "b_"),
            }

            for bi in range(bpc):
                p.sig_insts = []
                p.sz_dram = {
                    pre: dram_pool.tile([DI, L], FP16, tag=f"sz_dram{pre}{bi % 2}",
                                        name=f"sz_dram{pre}")
                    for pre in ("f_", "b_")
                }
                outf_dram = dram_pool.tile([L, D], FP16, tag=f"outf_dram{bi % 2}",
                                           name="outf_dram")
                p.xT = [
                    io_pool.tile([128, L], FP16, tag=f"xT{k}", name=f"xT{k}")
                    for k in range(4)
                ]
                for k in range(4):
                    nc.sync.dma_start(
                        p.xT[k][:], xb_d[bi * D + 128 * k: bi * D + 128 * (k + 1), :]
                    )

                ten_f = _phase_a(nc, p, "f_", rev=False, bi=bi)
                ten_b = _phase_a(nc, p, "b_", rev=True, bi=bi)

                def emit_f(m, po):
                    st = p.work_pool.tile([128, D], FP16, tag="outf_st", name="outf_st", bufs=2)
                    nc.scalar.activation(st[:], po[:], AF.Copy)
                    nc.sync.dma_start(outf_dram[128 * m:128 * (m + 1), :], st[:])

                def emit_b(m, po, bi=bi):
                    # combine (f + b)/2 + x, then layernorm over D, then store.
                    # x in [t, D] layout comes from PE-transposing the xT tiles
                    # (out[i,j] = sum_d xT[d, i] * ident[d, j] = x[t=i, d=j]);
                    # the ps "mm" tag is free during phase D.
                    ps_x = p.ps_pool.tile([128, 512], F32, tag="mm", name="mm")
                    for k in range(4):
                        nc.tensor.matmul(
                            ps_x[:, 128 * k:128 * (k + 1)],
                            p.xT[k][:, 128 * m:128 * (m + 1)],
                            p.ident[:],
                            start=True, stop=True,
                        )
                    xnat_t = p.work_pool.tile([128, D], FP16, tag="xnat_t", name="xnat_t", bufs=2)
                    nc.scalar.activation(xnat_t[:], ps_x[:], AF.Copy)
                    outf = io_pool.tile([128, D], FP16, tag="outf_in", name="outf_in", bufs=1)
                    nc.sync.dma_start(outf[:], outf_dram[128 * m:128 * (m + 1), :])
                    pre_f = io_pool.tile([128, D], F32, tag="pre_f", name="pre_f", bufs=2)
                    nc.gpsimd.tensor_tensor(pre_f[:], outf[:], xnat_t[:], OP.add)
                    o = io_pool.tile([128, D], F32, tag="o_comb", name="o_comb", bufs=1)
                    mu_raw = s_pool.tile([128, 1], F32, tag="mu_raw", name="mu_raw")
                    nc.vector.scalar_tensor_tensor(
                        o[:], po[:], 1.0, pre_f[:], OP.mult, OP.add, accum_out=mu_raw[:]
                    )
                    mu = s_pool.tile([128, 1], F32, tag="mu", name="mu")
                    nc.vector.tensor_scalar(mu[:], mu_raw[:], 1.0 / D, None, OP.mult)
                    xm = io_pool.tile([128, D], F32, tag="xm", name="xm", bufs=2)
                    nc.vector.tensor_scalar(xm[:], o[:], mu[:, 0:1], None, OP.subtract)
                    sqd = io_pool.tile([128, D], F32, tag="pre_f", name="sqd", bufs=2)
                    var_raw = s_pool.tile([128, 1], F32, tag="var_raw", name="var_raw")
                    nc.scalar.activation(sqd[:], xm[:], AF.Square, accum_out=var_raw[:])
                    var = s_pool.tile([128, 1], F32, tag="var", name="var")
                    nc.vector.tensor_scalar(var[:], var_raw[:], 1.0 / D, None, OP.mult)
                    # rstd = exp(-0.5 * ln(var + eps)) — stays in the exp/ln table set
                    lv = s_pool.tile([128, 1], F32, tag="lv", name="lv")
                    nc.scalar.activation(lv[:], var[:], AF.Ln, bias=eps_t[:, 0:1])
                    rstd = s_pool.tile([128, 1], F32, tag="rstd", name="rstd")
                    nc.scalar.activation(rstd[:], lv[:], AF.Exp, scale=-0.5)
                    o1 = io_pool.tile([128, D], F32, tag="o_comb", name="o1", bufs=1)
                    nc.vector.scalar_tensor_tensor(
                        o1[:], xm[:], rstd[:, 0:1], g_rep[:, 0, :], OP.mult, OP.mult
                    )
                    o2 = io_pool.tile([128, D], FP16, tag="xnat_o", name="o2", bufs=2)
                    nc.gpsimd.tensor_tensor(o2[:], o1[:], bb_rep[:, 0, :], OP.add)
                    nc.sync.dma_start(
                        out_d[bi * L + 128 * m: bi * L + 128 * (m + 1), :], o2[:]
                    )

                _phase_bcd(nc, p, "f_", rev=False, ten=ten_f, emit_out=emit_f, bi=bi)
                _phase_bcd(nc, p, "b_", rev=True, ten=ten_b, emit_out=emit_b, bi=bi)

    nc.compile()
    return nc


_CACHE = {}


def _make_packs(inputs):
    host = {}
    for pre in ("f_", "b_"):
        host[pre + "w_inT"] = np.asarray(inputs[pre + "in_proj"], np.float32).T
        host[pre + "w_xT"] = np.asarray(inputs[pre + "x_proj"], np.float32).T
        host[pre + "w_dtT"] = np.asarray(inputs[pre + "dt_w"], np.float32).T
        host[pre + "w_outT"] = 0.5 * np.asarray(inputs[pre + "out_proj"], np.float32).T
        host[pre + "conv_w"] = np.asarray(inputs[pre + "conv_w"], np.float32)
        host[pre + "conv_b"] = np.asarray(inputs[pre + "conv_b"], np.float32).reshape(DI, 1)
        host[pre + "dt_b"] = np.asarray(inputs[pre + "dt_b"], np.float32).reshape(DI, 1)
        host[pre + "A"] = -np.exp(np.asarray(inputs[pre + "A_log"], np.float32))
        host[pre + "Dv"] = np.asarray(inputs[pre + "Dv"], np.float32).reshape(DI, 1)
    host["ident"] = np.eye(128, dtype=np.float32)
    bsel = np.zeros((64, 128 * 2 * NST), np.float32)
    for b in range(2 * NST):
        bsel[32 + b, 128 * b:128 * (b + 1)] = 1.0
    host["bsel"] = bsel
    host["ln_g"] = np.asarray(inputs["ln_g"], np.float32).reshape(1, D)
    host["ln_b"] = np.asarray(inputs["ln_b"], np.float32).reshape(1, D)

    wpack = np.empty(WN, nbf)
    for name, (off, shape) in WLAY.items():
        n = int(np.prod(shape))
        wpack[off:off + n] = np.ascontiguousarray(host[name]).astype(nbf).ravel()
    cpack = np.empty(CN, np.float32)
    for name, (off, shape) in CLAY.items():
        n = int(np.prod(shape))
        cpack[off:off + n] = np.ascontiguousarray(host[name]).ravel()
    return wpack, cpack


def _host_inputs(inputs, ncores=NCORES, bpc=BPC):
    """Per-core input maps (x only; weights are baked into the program)."""
    x = np.asarray(inputs["x"], np.float32)
    in_maps = []
    for i in range(ncores):
        xs = x[i * bpc:(i + 1) * bpc]  # (bpc, L, D)
        xb = np.ascontiguousarray(
            np.transpose(xs, (0, 2, 1)).reshape(bpc * D, L)
        ).astype(nbf)
        in_maps.append({"xb": xb})
    return in_maps


def _make_runner(nc, n_cores):
    """Compiled shard_map runner over the bass program; reusable across calls."""
    import jax
    from jax.sharding import Mesh, PartitionSpec
    from jax.experimental.shard_map import shard_map
    from concourse.bass2jax import (
        _bass_exec_p, install_neuronx_cc_hook, partition_id_tensor)

    install_neuronx_cc_hook()
    partition_name = nc.partition_id_tensor.name if nc.partition_id_tensor else None
    in_names, out_names, out_avals = [], [], []
    for alloc in nc.m.functions[0].allocations:
        if not isinstance(alloc, mybir.MemoryLocationSet):
            continue
        if alloc.kind == "ExternalInput":
            name = alloc.memorylocations[0].name
            if name != partition_name:
                in_names.append(name)
        elif alloc.kind == "ExternalOutput":
            out_names.append(alloc.memorylocations[0].name)
            out_avals.append(
                jax.core.ShapedArray(tuple(alloc.tensor_shape), mybir.dt.np(alloc.dtype))
            )
    n_params = len(in_names)
    all_names = in_names + out_names + ([partition_name] if partition_name else [])

    def _body(*args):
        operands = list(args)
        if partition_name is not None:
            operands.append(partition_id_tensor())
        return tuple(
            _bass_exec_p.bind(
                *operands,
                out_avals=tuple(out_avals),
                in_names=tuple(all_names),
                out_names=tuple(out_names),
                lowering_input_output_aliases=(),
                sim_require_finite=True,
                sim_require_nnan=True,
                nc=nc,
            )
        )

    devices = jax.devices()[:n_cores]
    mesh = Mesh(np.asarray(devices), ("core",))
    n_outs = len(out_names)
    sharded = jax.jit(
        shard_map(
            _body,
            mesh=mesh,
            in_specs=(PartitionSpec("core"),) * (n_params + n_outs),
            out_specs=(PartitionSpec("core"),) * n_outs,
            check_rep=False,
        ),
        keep_unused=True,
    )
    zeros = [
        np.zeros((n_cores * a.shape[0],) + tuple(a.shape[1:]), a.dtype)
        for a in out_avals
    ]

    def run(in_maps):
        import jax as _j

        concat_in = [
            np.concatenate([np.asarray(in_maps[c][nm]) for c in range(n_cores)], axis=0)
            for nm in in_names
        ]
        outs = sharded(*concat_in, *zeros)
        _j.block_until_ready(outs)
        return {nm: np.asarray(o) for nm, o in zip(out_names, outs)}

    def make_timed(in_maps):
        """Pre-stage inputs on device; return a closure that only executes.

        The returned callable returns the raw device outputs (no host
        conversion) so a timing loop measures execute cost only.
        """
        import jax as _jx

        concat_in = [
            np.concatenate([np.asarray(in_maps[c][nm]) for c in range(n_cores)], axis=0)
            for nm in in_names
        ]
        dev_in = [_jx.device_put(a) for a in concat_in + zeros]

        def timed_run():
            return sharded(*dev_in)

        return timed_run

    run.make_timed = make_timed
    return run


def _get_cached(inputs):
    wpack, cpack = _make_packs(inputs)
    key = (
        hashlib.sha1(wpack.tobytes()).hexdigest(),
        hashlib.sha1(cpack.tobytes()).hexdigest(),
    )
    if _CACHE.get("key") != key:
        _CACHE.clear()
        _CACHE["key"] = key
        _CACHE["nc"] = build_program(wpack, cpack)
        _CACHE["run"] = _make_runner(_CACHE["nc"], NCORES)
    return _CACHE["run"]


def kernel(**inputs):
    run = _get_cached(inputs)
    in_maps = _host_inputs(inputs)
    out = run(in_maps)["out"]
    return out.reshape(B, L, D).astype(np.float32)


# revision 24
# speedup vs baseline: 1.1577x; 1.1577x over previous
"""BiMambaBlock Trainium2 kernel.

Data-parallel over batch: BPC batches per core on NCORES cores
(NCORES * BPC = 8). Each core runs the full bidirectional Mamba block
per batch in one Bass program:

  - middle-section layout: [d_inner on partitions, time on free]
  - projections (in_proj / x_proj / dt_proj / out_proj) as PE GEMMs (fp16)
  - causal depthwise conv: fused mult-add (scalar_tensor_tensor) tap chain
  - silu / softplus fused into single ACT ops (AF.Silu, exp+ln)
  - selective scan via DVE tensor_tensor_scan, one lane per (d, n) pair;
    backward direction scans reversed-time APs
  - dBx/hC elementwise multiplies on GPSIMD (~1 in 5 dBx shifted to DVE
    to balance the two engines)
  - n-fold (sum_n C_n * h_n) via identity-matmul PSUM accumulation on PE
  - final combine + layernorm in [time on partitions, d_model on free]

I/O strategy: all weights/consts are baked into the executable as
HLO constants (nc.inline_tensor) so the axon PJRT tunnel does not
re-ship them on every execute; per-run traffic is one fp16 input
(x transposed, [bpc*D, L]) and one fp16 output ([bpc*L, D]). The
[L, D]-layout x needed for the residual+layernorm epilogue is
reconstructed on device by PE-transposing xb. All 16-bit tensors are
fp16 (not bf16): same size and engine rates, 3 more mantissa bits
(rel err 7e-4 vs 5e-3). The program cache is keyed on the weight
bytes, so a harness calling kernel() with different weights rebuilds
correctly.
"""

import hashlib
import sys

sys.path.insert(0, "/opt/trn_rl_repo")

import numpy as np

import concourse.bass as bass
import concourse.mybir as mybir
import concourse.tile as tile
from concourse import bacc

F32 = mybir.dt.float32
FP16 = mybir.dt.float16
AF = mybir.ActivationFunctionType
OP = mybir.AluOpType

B, L, D, DI, NST, RNK, KCONV = 8, 1024, 512, 1024, 16, 32, 4
LN_EPS = 1e-5
NB = DI // 128  # 8 d-blocks
TT = L // 128  # 8 time tiles
TCH = L // 512  # 2 matmul free chunks
PAD = KCONV - 1

BC_MODE = "dma"  # B/C broadcast: "dma" (partition-broadcast DMA) or "pe" (selector matmul)
NQ = 2  # n-steps per B/C broadcast DMA (dma mode)
DA_BUFS, DBX_BUFS, H_BUFS, HC_BUFS, XIP_BUFS = 2, 3, 3, 2, 2  # pipeline depths

NCORES = 4  # cores used (4 x bpc=2 beats 8 x bpc=1: the tunnel's per-core
# dispatch cost is strongly super-linear in core count, ~1.1 ms at 4 cores vs
# ~3.9 ms at 8, while the second batch per core adds only ~1.7 ms device time)
BPC = B // NCORES  # batches per core

nbf = np.float16


def _mk_layout(entries):
    off, lay = 0, {}
    for name, shape in entries:
        n = int(np.prod(shape))
        lay[name] = (off, shape)
        off += n
    return lay, off


WLAY, WN = _mk_layout(
    [(pre + nm, shp)
     for pre in ("f_", "b_")
     for nm, shp in (
         ("w_inT", (D, 2 * DI)),
         ("w_xT", (DI, 64)),
         ("w_dtT", (RNK, DI)),
         ("w_outT", (DI, D)),
     )]
    + [("ident", (128, 128)), ("bsel", (64, 128 * 2 * NST))]
)
CLAY, CN = _mk_layout(
    [(pre + nm, shp)
     for pre in ("f_", "b_")
     for nm, shp in (
         ("conv_w", (DI, KCONV)),
         ("conv_b", (DI, 1)),
         ("dt_b", (DI, 1)),
         ("A", (DI, NST)),
         ("Dv", (DI, 1)),
     )]
    + [("ln_g", (1, D)), ("ln_b", (1, D))]
)


class P:
    """Pool/handle bag shared by the phase builders."""


def _wap(p, name, r0, r1):
    """AP for rows [r0:r1) of packed weight tensor `name`."""
    off, (rows, cols) = WLAY[name]
    return p.wpack[off + r0 * cols: off + r1 * cols].rearrange(
        "(a b) -> a b", a=r1 - r0)


def _cap(p, name, r0, r1):
    off, (rows, cols) = CLAY[name]
    return p.cpack[off + r0 * cols: off + r1 * cols].rearrange(
        "(a b) -> a b", a=r1 - r0)


def _load_dir_consts(nc, p, pre):
    s_pool = p.s_pool
    h = {}
    h["conv_w"] = [s_pool.tile([128, KCONV], F32, tag=f"conv_w{pre}{m}", name=f"conv_w{pre}{m}") for m in range(NB)]
    h["conv_b"] = [s_pool.tile([128, 1], F32, tag=f"conv_b{pre}{m}", name=f"conv_b{pre}{m}") for m in range(NB)]
    h["dt_b"] = [s_pool.tile([128, 1], F32, tag=f"dt_b{pre}{m}", name=f"dt_b{pre}{m}") for m in range(NB)]
    h["a_sb"] = [s_pool.tile([128, NST], F32, tag=f"a_sb{pre}{m}", name=f"a_sb{pre}{m}") for m in range(NB)]
    h["dv"] = [s_pool.tile([128, 1], F32, tag=f"dv{pre}{m}", name=f"dv{pre}{m}") for m in range(NB)]
    for m in range(NB):
        r0, r1 = 128 * m, 128 * (m + 1)
        nc.sync.dma_start(h["conv_w"][m][:], _cap(p, pre + "conv_w", r0, r1))
        nc.sync.dma_start(h["conv_b"][m][:], _cap(p, pre + "conv_b", r0, r1))
        nc.sync.dma_start(h["dt_b"][m][:], _cap(p, pre + "dt_b", r0, r1))
        nc.sync.dma_start(h["a_sb"][m][:], _cap(p, pre + "A", r0, r1))
        nc.sync.dma_start(h["dv"][m][:], _cap(p, pre + "Dv", r0, r1))
    return h


def _phase_a(nc, p, pre, rev, bi):
    """in_proj GEMM; z -> silu(z) -> DRAM; xi -> causal conv -> silu -> xc."""
    w_in = [p.w_pool.tile([128, 2 * DI], FP16, tag=f"w_in{k}", name=f"w_in{k}") for k in range(4)]
    for k in range(4):
        nc.sync.dma_start(w_in[k][:], _wap(p, pre + "w_inT", 128 * k, 128 * (k + 1)))
    con = p.consts[pre]

    xc = [p.big_pool.tile([128, L], FP16, tag=f"{pre}xc{m}", name=f"{pre}xc{m}") for m in range(NB)]
    sz_dram = p.sz_dram[pre]

    # z tiles first (keeps all silu ACT ops before any exp/ln ACT ops)
    for m in range(2 * NB):
        mm = m + NB if m < NB else m - NB  # z tiles (8..15) first, then xi (0..7)
        xi_pad = None
        sz_st = None
        if mm < NB:
            xi_pad = p.work_pool.tile([128, L + PAD], FP16, tag="xi_pad", name="xi_pad", bufs=XIP_BUFS)
            if rev:
                nc.vector.memset(xi_pad[:, L:L + PAD], 0.0)
            else:
                nc.vector.memset(xi_pad[:, 0:PAD], 0.0)
        else:
            sz_st = p.work_pool.tile([128, L], FP16, tag="sz_st", name="sz_st", bufs=2)
        for tch in range(TCH):
            ps = p.ps_pool.tile([128, 512], F32, tag="mm", name="mm")
            for k in range(4):
                nc.tensor.matmul(
                    ps[:],
                    w_in[k][:, 128 * mm:128 * (mm + 1)],
                    p.xT[k][:, 512 * tch:512 * (tch + 1)],
                    start=(k == 0),
                    stop=(k == 3),
                )
            if mm < NB:
                off = (0 if rev else PAD) + 512 * tch
                nc.scalar.activation(xi_pad[:, off:off + 512], ps[:], AF.Copy)
            else:
                # silu(z) fused on ACT straight from PSUM
                p.sig_insts.append(
                    nc.scalar.activation(
                        sz_st[:, 512 * tch:512 * (tch + 1)], ps[:], AF.Silu
                    )
                )
        if mm < NB:
            # conv: fwd out[t] = sum_j w_j*xi[t-3+j]; bwd out[t] = sum_j w_j*xi[t+3-j]
            cw = con["conv_w"][mm]
            cb = con["conv_b"][mm]
            offs = [3 - j for j in range(KCONV)] if rev else list(range(KCONV))
            acc = [
                p.work_pool.tile([128, L], FP16, tag=f"cacc{j % 2}", name=f"cacc{j}", bufs=2)
                for j in range(2)
            ]
            # tap 0 on ACT (Copy with scale-ptr); taps 1-3 as fused mult-add on DVE
            nc.scalar.activation(
                acc[0][:], xi_pad[:, offs[0]:offs[0] + L], AF.Copy, scale=cw[:, 0:1]
            )
            for j in range(1, KCONV):
                o = offs[j]
                nc.vector.scalar_tensor_tensor(
                    acc[j % 2][:], xi_pad[:, o:o + L], cw[:, j:j + 1], acc[(j - 1) % 2][:],
                    OP.mult, OP.add,
                )
            # xc = silu(acc + conv_b) fused on ACT
            p.sig_insts.append(
                nc.scalar.activation(xc[mm][:], acc[(KCONV - 1) % 2][:], AF.Silu, bias=cb[:, 0:1])
            )
        else:
            nc.sync.dma_start(sz_dram[128 * (mm - NB):128 * (mm - NB + 1), :], sz_st[:])
    return {"xc": xc}


def _phase_bcd(nc, p, pre, rev, ten, emit_out, bi):
    xc = ten["xc"]
    con = p.consts[pre]
    sz_dram = p.sz_dram[pre]
    w_x = [p.w_pool.tile([128, 64], FP16, tag=f"w_x{k}", name=f"w_x{k}") for k in range(NB)]
    for k in range(NB):
        nc.sync.dma_start(w_x[k][:], _wap(p, pre + "w_xT", 128 * k, 128 * (k + 1)))
    w_dt = p.w_pool.tile([RNK, DI], FP16, tag="w_dt", name="w_dt")
    nc.sync.dma_start(w_dt[:], _wap(p, pre + "w_dtT", 0, RNK))
    w_out = [p.w_pool.tile([128, D], FP16, tag=f"w_out{k}", name=f"w_out{k}") for k in range(NB)]
    for k in range(NB):
        nc.sync.dma_start(w_out[k][:], _wap(p, pre + "w_outT", 128 * k, 128 * (k + 1)))

    # --- phase B: x_proj -> (dt | B | C); dt_proj -> delta ---
    dbl_tag = f"{pre}dbl" if BC_MODE == "pe" else "dbl"
    dbl = p.big_pool.tile([64, L], FP16, tag=dbl_tag, name=f"{pre}dbl")
    for tch in range(TCH):
        ps = p.ps_pool.tile([64, 512], F32, tag="mm", name="mm")
        for k in range(NB):
            nc.tensor.matmul(
                ps[:], w_x[k][:], xc[k][:, 512 * tch:512 * (tch + 1)],
                start=(k == 0), stop=(k == NB - 1),
            )
        nc.scalar.activation(dbl[:, 512 * tch:512 * (tch + 1)], ps[:], AF.Copy)
    if BC_MODE == "dma":
        # bc_dram[n, 0, :] = B_n, bc_dram[n, 1, :] = C_n
        bc_dram = p.dram_pool.tile([NST, 2, L], FP16, tag=f"bc_dram{pre}{bi % 2}",
                                   name="bc_dram")
        nc.sync.dma_start(bc_dram[:, 0, :], dbl[RNK:RNK + NST, :])
        nc.sync.dma_start(bc_dram[:, 1, :], dbl[RNK + NST:RNK + 2 * NST, :])

    delta = [p.big_pool.tile([128, L], FP16, tag=f"delta{m}", name=f"delta{m}") for m in range(NB)]
    for m in range(NB):
        for tch in range(TCH):
            ps = p.ps_pool.tile([128, 512], F32, tag="mm", name="mm")
            nc.tensor.matmul(
                ps[:],
                w_dt[:, 128 * m:128 * (m + 1)],
                dbl[0:RNK, 512 * tch:512 * (tch + 1)],
                start=True, stop=True,
            )
            # softplus(s) = ln(1 + e^s) via the exp/ln table set
            spu = p.work_pool.tile([128, 512], F32, tag="spu", name="spu", bufs=1)
            nc.scalar.activation(spu[:], ps[:], AF.Exp, bias=con["dt_b"][m][:, 0:1])
            nc.scalar.activation(
                delta[m][:, 512 * tch:512 * (tch + 1)], spu[:], AF.Ln, bias=1.0
            )

    # --- phase C: selective scan + n-fold + gate ---
    yg = [p.big_pool.tile([128, L], FP16, tag=f"yg{m}", name=f"yg{m}") for m in range(NB)]
    for g in range(NB // 2):
        yp = [p.psy_pool.tile([128, L], F32, tag=f"yp{d2}", name=f"yp{d2}") for d2 in range(2)]
        dtx = [
            p.work_pool.tile([128, L], FP16, tag=f"cacc{d2}", name=f"dtx{d2}", bufs=2)
            for d2 in range(2)
        ]
        for d2 in range(2):
            m = 2 * g + d2
            nc.gpsimd.tensor_tensor(dtx[d2][:], delta[m][:], xc[m][:], OP.mult)
        def emit_bc(n):
            # broadcast B_n / C_n rows of dbl across 128 partitions on PE
            # (selector matmul), then ACT-copy PSUM -> SBUF.
            bcBt = p.w_pool.tile([128, L], FP16, tag="bcB", name="bcB", bufs=2)
            bcCt = p.w_pool.tile([128, L], FP16, tag="bcC", name="bcC", bufs=2)
            for dst, b in ((bcBt, n), (bcCt, NST + n)):
                for tch in range(TCH):
                    psb = p.ps_pool.tile([128, 512], F32, tag="mm", name="mm")
                    nc.tensor.matmul(
                        psb[:],
                        p.bsel[:, 128 * b:128 * (b + 1)],
                        dbl[0:64, 512 * tch:512 * (tch + 1)],
                        start=True, stop=True,
                    )
                    nc.scalar.activation(
                        dst[:, 512 * tch:512 * (tch + 1)], psb[:], AF.Copy
                    )
            return bcBt, bcCt

        if BC_MODE == "pe":
            bc_cur = emit_bc(0)
        bcq = None
        for n in range(NST):
            if BC_MODE == "pe":
                # prefetch next n's broadcast BEFORE this n's consumers so the
                # in-order PE/ACT streams issue it ahead of the yp-fold stall
                bc_next = emit_bc(n + 1) if n + 1 < NST else None
                bcB = bc_cur[0][:]
                bcC = bc_cur[1][:]
            else:
                if n % NQ == 0:
                    bcq = p.w_pool.tile(
                        [128, NQ, 2, L], FP16, tag=f"bc{(n // NQ) % 2}", name="bc",
                        bufs=1,
                    )
                    nc.sync.dma_start(
                        bcq[:], bc_dram[n:n + NQ, :, :].partition_broadcast(128)
                    )
                bcB = bcq[:, n % NQ, 0, :]
                bcC = bcq[:, n % NQ, 1, :]
            for d2 in range(2):
                m = 2 * g + d2
                da = p.work_pool.tile([128, L], FP16, tag=f"da{d2}", name="da", bufs=DA_BUFS)
                nc.scalar.activation(
                    da[:], delta[m][:], AF.Exp, scale=con["a_sb"][m][:, n:n + 1]
                )
                dbx = p.work_pool.tile([128, L], FP16, tag=f"dbx{d2}", name="dbx", bufs=DBX_BUFS)
                # GPSIMD carries the dbx/hc multiplies, but it is the busiest
                # engine (457us vs DVE 387us); shifting ~1 in 5 dbx ops to DVE
                # balances the two at ~416us each.
                dbx_eng = nc.vector if (g * NST + n) % 5 == 0 else nc.gpsimd
                dbx_eng.tensor_tensor(dbx[:], dtx[d2][:], bcB, OP.mult)
                h = p.work_pool.tile([128, L], FP16, tag=f"h{d2}", name="h", bufs=H_BUFS)
                if rev:
                    nc.vector.tensor_tensor_scan(
                        h[:, ::-1], da[:, ::-1], dbx[:, ::-1], 0.0, OP.mult, OP.add
                    )
                else:
                    nc.vector.tensor_tensor_scan(h[:], da[:], dbx[:], 0.0, OP.mult, OP.add)
                hc = p.work_pool.tile([128, L], FP16, tag=f"hc{d2}", name="hc", bufs=HC_BUFS)
                nc.gpsimd.tensor_tensor(hc[:], h[:], bcC, OP.mult)
                for tch in range(TCH):
                    nc.tensor.matmul(
                        yp[d2][:, 512 * tch:512 * (tch + 1)],
                        p.ident[:],
                        hc[:, 512 * tch:512 * (tch + 1)],
                        start=(n == 0), stop=(n == NST - 1),
                    )
            if BC_MODE == "pe":
                bc_cur = bc_next
        # gate: yg = (y + xc*Dv) * silu(z);  silu(z) streamed back from DRAM
        for d2 in range(2):
            m = 2 * g + d2
            szin = p.io_pool.tile([128, L], FP16, tag=f"szin{d2}", name="szin")
            nc.sync.dma_start(szin[:], sz_dram[128 * m:128 * (m + 1), :])
            for tch in range(TCH):
                nc.vector.scalar_tensor_tensor(
                    yg[m][:, 512 * tch:512 * (tch + 1)],
                    xc[m][:, 512 * tch:512 * (tch + 1)],
                    con["dv"][m][:, 0:1],
                    yp[d2][:, 512 * tch:512 * (tch + 1)],
                    OP.mult, OP.add,
                )
            nc.vector.tensor_tensor(yg[m][:], yg[m][:], szin[:], OP.mult)

    # --- phase D: out_proj GEMM -> [t, D] PSUM tiles ---
    for m in range(TT):
        po = p.psd_pool.tile([128, D], F32, tag="po", name="po")
        for k in range(NB):
            nc.tensor.matmul(
                po[:], yg[k][:, 128 * m:128 * (m + 1)], w_out[k][:],
                start=(k == 0), stop=(k == NB - 1),
            )
        emit_out(m, po)


def build_program(wpack_np, cpack_np, bpc=BPC):
    nc = bacc.Bacc("TRN2", target_bir_lowering=False, debug=False)

    # Force exp/ln onto the one table set that has BOTH, so softplus
    # (exp then ln) doesn't ping-pong table loads. List order (= set ids)
    # is preserved; we only hide exp/ln from the other sets.
    import concourse.bacc as _bacc_mod
    from concourse.hw_specs import get_activation_tables as _gat

    def _patched_tables():
        tables = list(_gat(nc.m.arch).items())
        out = []
        for name, s in tables:
            if name != "natural_log_exp_and_others":
                s = s - {AF.Exp, AF.Ln}
            out.append((name, s))
        _bacc_mod._bass_rust.insert_act_table_loads(nc, out)

    nc.insert_act_table_loads = _patched_tables

    wpack_d = nc.inline_tensor(wpack_np, "wpack")
    cpack_d = nc.inline_tensor(cpack_np, "cpack")
    xb_d = nc.dram_tensor("xb", [bpc * D, L], FP16, kind="ExternalInput")
    out_d = nc.dram_tensor("out", [bpc * L, D], FP16, kind="ExternalOutput")

    with tile.TileContext(nc) as tc:
        with (
            tc.tile_pool(name="io", bufs=1) as io_pool,
            tc.tile_pool(name="w", bufs=1) as w_pool,
            tc.tile_pool(name="big", bufs=1) as big_pool,
            tc.tile_pool(name="work", bufs=2) as work_pool,
            tc.tile_pool(name="s", bufs=1) as s_pool,
            tc.tile_pool(name="ps", bufs=2, space="PSUM") as ps_pool,
            tc.tile_pool(name="psy", bufs=1, space="PSUM") as psy_pool,
            tc.tile_pool(name="psd", bufs=2, space="PSUM") as psd_pool,
            tc.tile_pool(name="dram", bufs=1, space="DRAM") as dram_pool,
        ):
            p = P()
            p.io_pool, p.w_pool, p.big_pool, p.work_pool, p.s_pool = (
                io_pool, w_pool, big_pool, work_pool, s_pool)
            p.ps_pool, p.psy_pool, p.psd_pool, p.dram_pool = (
                ps_pool, psy_pool, psd_pool, dram_pool)
            p.wpack, p.cpack = wpack_d, cpack_d

            p.ident = io_pool.tile([128, 128], FP16, tag="ident", name="ident")
            nc.sync.dma_start(p.ident[:], _wap(p, "ident", 0, 128))
            # broadcast selector: sel[k, 128*b + i] = 1 iff k == 32 + b, so
            # sel[:, 128b:128(b+1)].T @ dbl[0:64, :] replicates dbl row 32+b
            # (B_n for b = n, C_n for b = 16 + n) across all 128 partitions.
            if BC_MODE == "pe":
                p.bsel = io_pool.tile([64, 128 * 2 * NST], FP16, tag="bsel", name="bsel")
                nc.sync.dma_start(p.bsel[:], _wap(p, "bsel", 0, 64))
            g_rep = io_pool.tile([128, 1, D], F32, tag="g_rep", name="g_rep")
            bb_rep = io_pool.tile([128, 1, D], F32, tag="bb_rep", name="bb_rep")
            nc.sync.dma_start(g_rep[:], _cap(p, "ln_g", 0, 1).partition_broadcast(128))
            nc.sync.dma_start(bb_rep[:], _cap(p, "ln_b", 0, 1).partition_broadcast(128))
            eps_t = s_pool.tile([128, 1], F32, tag="eps_t", name="eps_t")
            nc.gpsimd.memset(eps_t[:], LN_EPS)

            p.consts = {
                "f_": _load_dir_consts(nc, p, "f_"),
                "b_": _load_dir_consts(nc, p, "b_"),
            }

            for bi in range(bpc):
                p.sig_insts = []
                p.sz_dram = {
                    pre: dram_pool.tile([DI, L], FP16, tag=f"sz_dram{pre}{bi % 2}",
                                        name=f"sz_dram{pre}")
                    for pre in ("f_", "b_")
                }
                outf_dram = dram_pool.tile([L, D], FP16, tag=f"outf_dram{bi % 2}",
                                           name="outf_dram")
                p.xT = [
                    io_pool.tile([128, L], FP16, tag=f"xT{k}", name=f"xT{k}")
                    for k in range(4)
                ]
                for k in range(4):
                    nc.sync.dma_start(
                        p.xT[k][:], xb_d[bi * D + 128 * k: bi * D + 128 * (k + 1), :]
                    )

                ten_f = _phase_a(nc, p, "f_", rev=False, bi=bi)
                ten_b = _phase_a(nc, p, "b_", rev=True, bi=bi)

                def emit_f(m, po):
                    st = p.work_pool.tile([128, D], FP16, tag="outf_st", name="outf_st", bufs=2)
                    nc.scalar.activation(st[:], po[:], AF.Copy)
                    nc.sync.dma_start(outf_dram[128 * m:128 * (m + 1), :], st[:])

                def emit_b(m, po, bi=bi):
                    # combine (f + b)/2 + x, then layernorm over D, then store.
                    # x in [t, D] layout comes from PE-transposing the xT tiles
                    # (out[i,j] = sum_d xT[d, i] * ident[d, j] = x[t=i, d=j]);
                    # the ps "mm" tag is free during phase D.
                    ps_x = p.ps_pool.tile([128, 512], F32, tag="mm", name="mm")
                    for k in range(4):
                        nc.tensor.matmul(
                            ps_x[:, 128 * k:128 * (k + 1)],
                            p.xT[k][:, 128 * m:128 * (m + 1)],
                            p.ident[:],
                            start=True, stop=True,
                        )
                    xnat_t = p.work_pool.tile([128, D], FP16, tag="xnat_t", name="xnat_t", bufs=2)
                    nc.scalar.activation(xnat_t[:], ps_x[:], AF.Copy)
                    outf = io_pool.tile([128, D], FP16, tag="outf_in", name="outf_in", bufs=1)
                    nc.sync.dma_start(outf[:], outf_dram[128 * m:128 * (m + 1), :])
                    pre_f = io_pool.tile([128, D], F32, tag="pre_f", name="pre_f", bufs=2)
                    nc.gpsimd.tensor_tensor(pre_f[:], outf[:], xnat_t[:], OP.add)
                    o = io_pool.tile([128, D], F32, tag="o_comb", name="o_comb", bufs=1)
                    mu_raw = s_pool.tile([128, 1], F32, tag="mu_raw", name="mu_raw")
                    nc.vector.scalar_tensor_tensor(
                        o[:], po[:], 1.0, pre_f[:], OP.mult, OP.add, accum_out=mu_raw[:]
                    )
                    mu = s_pool.tile([128, 1], F32, tag="mu", name="mu")
                    nc.vector.tensor_scalar(mu[:], mu_raw[:], 1.0 / D, None, OP.mult)
                    xm = io_pool.tile([128, D], F32, tag="xm", name="xm", bufs=2)
                    nc.vector.tensor_scalar(xm[:], o[:], mu[:, 0:1], None, OP.subtract)
                    sqd = io_pool.tile([128, D], F32, tag="pre_f", name="sqd", bufs=2)
                    var_raw = s_pool.tile([128, 1], F32, tag="var_raw", name="var_raw")
                    nc.scalar.activation(sqd[:], xm[:], AF.Square, accum_out=var_raw[:])
                    var = s_pool.tile([128, 1], F32, tag="var", name="var")
                    nc.vector.tensor_scalar(var[:], var_raw[:], 1.0 / D, None, OP.mult)
                    # rstd = exp(-0.5 * ln(var + eps)) — stays in the exp/ln table set
                    lv = s_pool.tile([128, 1], F32, tag="lv", name="lv")
                    nc.scalar.activation(lv[:], var[:], AF.Ln, bias=eps_t[:, 0:1])
                    rstd = s_pool.tile([128, 1], F32, tag="rstd", name="rstd")
                    nc.scalar.activation(rstd[:], lv[:], AF.Exp, scale=-0.5)
                    o1 = io_pool.tile([128, D], F32, tag="o_comb", name="o1", bufs=1)
                    nc.vector.scalar_tensor_tensor(
                        o1[:], xm[:], rstd[:, 0:1], g_rep[:, 0, :], OP.mult, OP.mult
                    )
                    o2 = io_pool.tile([128, D], FP16, tag="xnat_o", name="o2", bufs=2)
                    nc.gpsimd.tensor_tensor(o2[:], o1[:], bb_rep[:, 0, :], OP.add)
                    nc.sync.dma_start(
                        out_d[bi * L + 128 * m: bi * L + 128 * (m + 1), :], o2[:]
                    )

                _phase_bcd(nc, p, "f_", rev=False, ten=ten_f, emit_out=emit_f, bi=bi)
                _phase_bcd(nc, p, "b_", rev=True, ten=ten_b, emit_out=emit_b, bi=bi)

    nc.compile()
    return nc


_CACHE = {}


def _make_packs(inputs):
    host = {}
    for pre in ("f_", "b_"):
        host[pre + "w_inT"] = np.asarray(inputs[pre + "in_proj"], np.float32).T
        host[pre + "w_xT"] = np.asarray(inputs[pre + "x_proj"], np.float32).T
        host[pre + "w_dtT"] = np.asarray(inputs[pre + "dt_w"], np.float32).T
        host[pre + "w_outT"] = 0.5 * np.asarray(inputs[pre + "out_proj"], np.float32).T
        host[pre + "conv_w"] = np.asarray(inputs[pre + "conv_w"], np.float32)
        host[pre + "conv_b"] = np.asarray(inputs[pre + "conv_b"], np.float32).reshape(DI, 1)
        host[pre + "dt_b"] = np.asarray(inputs[pre + "dt_b"], np.float32).reshape(DI, 1)
        host[pre + "A"] = -np.exp(np.asarray(inputs[pre + "A_log"], np.float32))
        host[pre + "Dv"] = np.asarray(inputs[pre + "Dv"], np.float32).reshape(DI, 1)
    host["ident"] = np.eye(128, dtype=np.float32)
    bsel = np.zeros((64, 128 * 2 * NST), np.float32)
    for b in range(2 * NST):
        bsel[32 + b, 128 * b:128 * (b + 1)] = 1.0
    host["bsel"] = bsel
    host["ln_g"] = np.asarray(inputs["ln_g"], np.float32).reshape(1, D)
    host["ln_b"] = np.asarray(inputs["ln_b"], np.float32).reshape(1, D)

    wpack = np.empty(WN, nbf)
    for name, (off, shape) in WLAY.items():
        n = int(np.prod(shape))
        wpack[off:off + n] = np.ascontiguousarray(host[name]).astype(nbf).ravel()
    cpack = np.empty(CN, np.float32)
    for name, (off, shape) in CLAY.items():
        n = int(np.prod(shape))
        cpack[off:off + n] = np.ascontiguousarray(host[name]).ravel()
    return wpack, cpack


def _host_inputs(inputs, ncores=NCORES, bpc=BPC):
    """Per-core input maps (x only; weights are baked into the program)."""
    x = np.asarray(inputs["x"], np.float32)
    in_maps = []
    for i in range(ncores):
        xs = x[i * bpc:(i + 1) * bpc]  # (bpc, L, D)
        xb = np.ascontiguousarray(
            np.transpose(xs, (0, 2, 1)).reshape(bpc * D, L)
        ).astype(nbf)
        in_maps.append({"xb": xb})
    return in_maps


def _make_runner(nc, n_cores):
    """Compiled shard_map runner over the bass program; reusable across calls."""
    import jax
    from jax.sharding import Mesh, PartitionSpec
    from jax.experimental.shard_map import shard_map
    from concourse.bass2jax import (
        _bass_exec_p, install_neuronx_cc_hook, partition_id_tensor)

    install_neuronx_cc_hook()
    partition_name = nc.partition_id_tensor.name if nc.partition_id_tensor else None
    in_names, out_names, out_avals = [], [], []
    for alloc in nc.m.functions[0].allocations:
        if not isinstance(alloc, mybir.MemoryLocationSet):
            continue
        if alloc.kind == "ExternalInput":
            name = alloc.memorylocations[0].name
            if name != partition_name:
                in_names.append(name)
        elif alloc.kind == "ExternalOutput":
            out_names.append(alloc.memorylocations[0].name)
            out_avals.append(
                jax.core.ShapedArray(tuple(alloc.tensor_shape), mybir.dt.np(alloc.dtype))
            )
    n_params = len(in_names)
    all_names = in_names + out_names + ([partition_name] if partition_name else [])

    def _body(*args):
        operands = list(args)
        if partition_name is not None:
            operands.append(partition_id_tensor())
        return tuple(
            _bass_exec_p.bind(
                *operands,
                out_avals=tuple(out_avals),
                in_names=tuple(all_names),
                out_names=tuple(out_names),
                lowering_input_output_aliases=(),
                sim_require_finite=True,
                sim_require_nnan=True,
                nc=nc,
            )
        )

    devices = jax.devices()[:n_cores]
    mesh = Mesh(np.asarray(devices), ("core",))
    n_outs = len(out_names)
    sharded = jax.jit(
        shard_map(
            _body,
            mesh=mesh,
            in_specs=(PartitionSpec("core"),) * (n_params + n_outs),
            out_specs=(PartitionSpec("core"),) * n_outs,
            check_rep=False,
        ),
        keep_unused=True,
    )
    zeros = [
        np.zeros((n_cores * a.shape[0],) + tuple(a.shape[1:]), a.dtype)
        for a in out_avals
    ]

    def run(in_maps):
        import jax as _j

        concat_in = [
            np.concatenate([np.asarray(in_maps[c][nm]) for c in range(n_cores)], axis=0)
            for nm in in_names
        ]
        outs = sharded(*concat_in, *zeros)
        _j.block_until_ready(outs)
        return {nm: np.asarray(o) for nm, o in zip(out_names, outs)}

    def make_timed(in_maps):
        """Pre-stage inputs on device; return a closure that only executes.

        The returned callable returns the raw device outputs (no host
        conversion) so a timing loop measures execute cost only.
        """
        import jax as _jx

        concat_in = [
            np.concatenate([np.asarray(in_maps[c][nm]) for c in range(n_cores)], axis=0)
            for nm in in_names
        ]
        dev_in = [_jx.device_put(a) for a in concat_in + zeros]

        def timed_run():
            return sharded(*dev_in)

        return timed_run

    run.make_timed = make_timed
    return run


def _get_cached(inputs):
    wpack, cpack = _make_packs(inputs)
    key = (
        hashlib.sha1(wpack.tobytes()).hexdigest(),
        hashlib.sha1(cpack.tobytes()).hexdigest(),
    )
    if _CACHE.get("key") != key:
        _CACHE.clear()
        _CACHE["key"] = key
        _CACHE["nc"] = build_program(wpack, cpack)
        _CACHE["run"] = _make_runner(_CACHE["nc"], NCORES)
    return _CACHE["run"]


def kernel(**inputs):
    run = _get_cached(inputs)
    in_maps = _host_inputs(inputs)
    out = run(in_maps)["out"]
    return out.reshape(B, L, D).astype(np.float32)
